# revision 48
# baseline (speedup 1.0000x reference)
"""Trainium2 Bass kernel for nn_DenseRED_SN (per-pixel spectral-norm dense reduce).

Math (full problem):
    w_mat = weight.reshape(H*W, C)
    sigma[p]  = ||w_mat[p, :]||_2                       (per-pixel L2 norm)
    out[b, 0, p] = (sum_c x[b, c, p] * w_mat[p, c]) / sigma[p] + bias[p]

Sharding: pixel-parallel over H across the 8 cores (32 image rows each).
Each core's slice of x / weight / bias is host-repacked (layout + fp16
staging cast) into an SBUF-friendly "channel + pixel-half on partitions"
layout:
    partition p = h*64 + c   (h = pixel-half 0/1, c = channel)
    x_core[b, p, f]  = x[b, c, pix]  with pix = h*4096 + f
    w_core[p, f]     = w_mat[pix, c]

The kernel is HBM-bound; x/w are staged as fp16 (harness tolerance 2e-2,
this lands ~6e-4), halving the dominant traffic: 18MB/core at the
~358GB/s HBM-per-core limit.  Both HWDGE rings (SP + ACT) stream
concurrently: every batch's column half A rides SP and half B rides ACT,
so a full batch lands every ~2.6us and VectorE — whose multiply stream
(~41us) nearly fills the ~46us DMA stream — is fed smoothly.

On-chip per core (all arithmetic on device):
    sq    = w ⊙ w                                  (VectorE, fills pre-x0 idle)
    sig2  = ones_rep.T @ sq                        (PE, subgroup layout)
    rsig  = rsqrt(sig2)                            (ScalarE Abs_reciprocal_sqrt,
                                                    dispatched behind early descgens)
    per batch b: prod = x_b ⊙ w                    (VectorE f16 2x mode)
                 acc_j += ones_blk_b.T @ prod      (PE f16, accumulating)
    out_f16 = acc_j ⊙ rsig (+ bias)                (VectorE PSUM drain+scale,
                                                    per column half, each half's
                                                    store on its own ring)

PSUM layout: output row 32s+2b+h for column subgroup s (cols 1024s..)
lives on PSUM partitions 32s.., written via matmul tile_position=(0,32s);
every FD-bound op runs at FD<=1024 on 128 partitions.  The accumulator is
SPLIT into two single-bank [128, 512] tiles (one per within-subgroup
column half j) so draining half 0 doesn't WAR-serialize against the last
matmul of half 1.  The last two batches stream in 1024-col pieces with
matmuls riding each piece, so the final piece -> matmul -> drain -> store
chain is ~4us after the last x byte.

Scheduling invariants learned on HW (keep these when editing):
  - Engine-op dispatches block that engine's sequencer; the Scalar
    sequencer doubles as the ACT ring's descriptor generator, so bulk
    ScalarE work or early-waiting dispatches starve half the DMA stream.
  - GpSimd and VectorE arbitrate one SBUF port pair with an exclusive
    instruction-length lock — do not offload elementwise work to gpsimd.
  - 0.25MB DMA pieces measurably stretch the stream; use them only where
    the tail latency win exceeds the stream cost (the last two batches).
"""

import os

import numpy as np

H, W, C, B = 256, 256, 64, 16
NCORES = 8
ROWS = H // NCORES        # 32 image rows per core
PIX = ROWS * W            # 8192 pixels per core
HALF = PIX // 2           # 4096 (free-dim size; two pixel halves on partitions)
NCHUNK = 512              # matmul moving free dim (one PSUM bank of fp32)
SUB = 1024                # columns per partition-subgroup in the PSUM layout
NSUB = HALF // SUB        # 4 subgroups -> partition blocks 0/32/64/96
X_BUFS = 8

_cache = {}


def _ensure_jax_platform():
    # bass2jax executes through the axon PJRT backend; make sure a
    # JAX_PLATFORMS=cpu pin from a caller does not hide the neuron devices.
    plat = os.environ.get("JAX_PLATFORMS")
    if plat is not None and "axon" not in plat and "neuron" not in plat:
        del os.environ["JAX_PLATFORMS"]


def _build_nc(use_f32r=True, with_bias=True):
    import concourse.bass as bass
    import concourse.tile as tile
    from concourse import bacc, mybir

    f32 = mybir.dt.float32
    f32r = mybir.dt.float32r
    f16 = mybir.dt.float16

    # Bacc (not raw Bass): its compile() pass lowers multi-wait instructions
    # into event-semaphore/NOP form — the raw 64B ISA slots hold only one
    # sync wait, so a plain Bass build fails walrus codegen on any
    # double-buffered pipeline.
    nc = bacc.Bacc("TRN2", target_bir_lowering=False, debug=False)

    # x and w are staged to device DRAM as fp16 (host-side cast): the
    # kernel is HBM-bound and the 2e-2 harness tolerance leaves ~40x
    # headroom over the ~5e-4 error fp16 staging introduces.  This
    # halves the dominant x traffic (33.5 MB -> 16.8 MB per core).
    x_d = nc.dram_tensor("x", [B, 128, HALF], f16, kind="ExternalInput")
    w_d = nc.dram_tensor("w", [128, HALF], f16, kind="ExternalInput")
    cdt = mybir.dt.float16 if use_f32r else f32
    oblk_d = nc.dram_tensor("ones_blk", [128, B * 32], cdt, kind="ExternalInput")
    orep_d = nc.dram_tensor("ones_rep", [128, 32], cdt, kind="ExternalInput")
    if with_bias:
        # host pre-packs bias (pure layout) into the [128, SUB] output layout
        bias_d = nc.dram_tensor("bias", [128, SUB], f16, kind="ExternalInput")
    # f16 output store: halves the tail store DMA; ~5e-4 relative error
    # added, far inside the harness tolerance
    out_d = nc.dram_tensor("out", [128, SUB], f16, kind="ExternalOutput")

    with tile.TileContext(nc) as tc:
        with (
            tc.tile_pool(name="const", bufs=1) as const_pool,
            tc.tile_pool(name="xin", bufs=X_BUFS) as x_pool,
            tc.tile_pool(name="prod", bufs=2) as prod_pool,
            tc.tile_pool(name="accp", bufs=1, space="PSUM") as acc_pool,
            tc.tile_pool(name="sigp", bufs=1, space="PSUM") as sig_pool,
        ):
            red_dt = f16 if use_f32r else f32

            # ---- constants / weight ----
            # w rides the ACT HWDGE ring in subgroup chunks: the first
            # batch's multiplies start as soon as their chunk of w (and of
            # x0) lands, instead of waiting for the whole 2MB.
            w_sb = const_pool.tile([128, HALF], f16)
            # w: two 0.25MB chunks per ring, so the first sq chunk (and the
            # whole sigma chain) starts ~1us earlier and is finished before
            # x0 lands — VectorE's pre-x0 idle absorbs the sigma work
            for s in range(NSUB):
                eng = nc.sync if s < 2 else nc.scalar
                eng.dma_start(
                    out=w_sb[:, s * SUB:(s + 1) * SUB],
                    in_=w_d[:, s * SUB:(s + 1) * SUB],
                )

            ones_blk = const_pool.tile([128, B, 32], red_dt)
            nc.gpsimd.dma_start(out=ones_blk[:], in_=oblk_d[:, :])
            ones_rep = const_pool.tile([128, 32], red_dt)
            nc.gpsimd.dma_start(out=ones_rep[:], in_=orep_d[:, :])
            if with_bias:
                bias_sb = const_pool.tile([128, SUB], f16)
                nc.gpsimd.dma_start(out=bias_sb[:], in_=bias_d[:, :])

            # sigma-chain tiles; the work itself is emitted inside the batch
            # loop (see emit_sigma) so it fills VectorE's early DMA-wait
            # gaps instead of delaying the first batch multiplies
            sq = prod_pool.tile([128, HALF], red_dt, tag="prod", name="sq")
            sig_ps = sig_pool.tile([128, SUB], f32)
            rsig = const_pool.tile([128, SUB], f32)

            def emit_sigma():
                # square on VectorE (f16 2x mode), per-w-chunk in arrival
                # order: it fills VectorE's idle window before x0 lands.
                # NOT on ScalarE — engine-op dispatches block the Scalar
                # sequencer (which doubles as ring B's descriptor
                # generator) until the ACT engine drains, starving ring B.
                for s in (0, 2, 1, 3):
                    nc.vector.tensor_mul(sq[:, s * SUB:(s + 1) * SUB],
                                         w_sb[:, s * SUB:(s + 1) * SUB],
                                         w_sb[:, s * SUB:(s + 1) * SUB])
                for s in range(NSUB):
                    for j in range(SUB // NCHUNK):
                        nc.tensor.matmul(
                            sig_ps[32 * s:32 * s + 32,
                                   j * NCHUNK:(j + 1) * NCHUNK],
                            ones_rep[:],
                            sq[:, s * SUB + j * NCHUNK:
                               s * SUB + (j + 1) * NCHUNK],
                            start=True,
                            stop=True,
                            tile_position=(0, 32 * s),
                        )

            # ---- main loop over batches: accumulate into PSUM ----
            # two separate accumulator tiles (one PSUM bank each, columns
            # j*512..): draining half 0 must not create a tile-level WAR
            # hazard against the last matmul still writing half 1
            out_sb = const_pool.tile([128, SUB], f16)
            acc_j = [acc_pool.tile([128, NCHUNK], f32, tag=f"acc{j}",
                                   name=f"acc{j}")
                     for j in range(SUB // NCHUNK)]

            def mm(b, c):
                s, j = divmod(c, SUB // NCHUNK)
                nc.tensor.matmul(
                    acc_j[j][32 * s:32 * s + 32, :],
                    ones_blk[:, b, :],
                    prods[b][:, c * NCHUNK:(c + 1) * NCHUNK],
                    start=(b == 0),
                    stop=False,
                    skip_group_check=True,
                    tile_position=(0, 32 * s),
                )

            def drain_half(j):
                # acc_j[j] is final once every batch's (s, j) chunk has been
                # matmul'd; scale by 1/sigma and store as f16.  Half 0
                # drains while half 1's last matmul runs.
                lo, hi = j * NCHUNK, (j + 1) * NCHUNK
                nc.vector.tensor_mul(out_sb[:, lo:hi], acc_j[j][:, :],
                                     rsig[:, lo:hi])
                if with_bias:
                    nc.vector.tensor_add(out_sb[:, lo:hi], out_sb[:, lo:hi],
                                         bias_sb[:, lo:hi])
                eng = nc.scalar if j == 0 else nc.sync
                eng.dma_start(out=out_d[:, lo:hi], in_=out_sb[:, lo:hi])

            emit_sigma()

            # Every batch is split across BOTH HWDGE rings: column half A
            # (cols 0..2047, PSUM subgroups 0-1) rides the SP ring, half B
            # (cols 2048.., subgroups 2-3) rides the ACT ring.  The rings
            # advance in lockstep, so a full batch lands every ~2.6us
            # (instead of two whole batches colliding every ~5.2us) and
            # VectorE is fed smoothly enough to enter the tail with no
            # backlog.
            prods = {}

            def dma_x(b, pieces_per_half):
                x_t = x_pool.tile([128, HALF], f16, tag="x", name=f"x_{b}")
                p = (HALF // 2) // pieces_per_half
                for h, eng in ((0, nc.sync), (1, nc.scalar)):
                    for v in range(pieces_per_half):
                        lo = h * (HALF // 2) + v * p
                        x_t_sl = x_t[:, lo:lo + p]
                        eng.dma_start(out=x_t_sl, in_=x_d[b, :, lo:lo + p])
                prods[b] = prod_pool.tile([128, HALF], red_dt, tag="prod",
                                          name=f"prod_{b}")
                return x_t

            def mul_piece(b, x_t, v, n_mul):
                w = HALF // n_mul
                nc.vector.tensor_mul(
                    prods[b][:, v * w:(v + 1) * w],
                    x_t[:, v * w:(v + 1) * w],
                    w_sb[:, v * w:(v + 1) * w],
                )

            for b in range(B - 1):
                if b == B - 2:
                    # second-to-last batch streams in 1024-col pieces so
                    # VectorE enters the final batch with less backlog
                    x_t = dma_x(b, 2)
                    for v in (0, 2, 1, 3):
                        mul_piece(b, x_t, v, NSUB)
                    for c in range(HALF // NCHUNK):
                        mm(b, c)
                    continue
                x_t = dma_x(b, 1)
                for v in range(2):        # half A then half B (same-time)
                    mul_piece(b, x_t, v, 2)
                if b == 3:
                    # 1/sigma in a single ScalarE op (|x|^-1/2; sigma^2>0 so
                    # abs is a no-op); emitted mid-loop so its sequencer
                    # wait sits behind a few DMA descgens instead of
                    # stalling the second ring
                    nc.scalar.activation(
                        out=rsig[:], in_=sig_ps[:],
                        func=mybir.ActivationFunctionType.Abs_reciprocal_sqrt,
                    )
                for c in range(HALF // NCHUNK):
                    mm(b, c)

            # ---- tail: last batch in 1024-col pieces, two per ring; the
            # final two pieces (one per ring) land together at stream end,
            # then piece->matmul->drain->store chains with the j=0 half's
            # store overlapping the j=1 half's drain.
            xb = dma_x(B - 1, 2)
            ARRIVAL = (0, 2, 1, 3)        # sync: pieces 0,1; scalar: 2,3
            for v in ARRIVAL[:-1]:
                mul_piece(B - 1, xb, v, NSUB)
                mm(B - 1, 2 * v)
                mm(B - 1, 2 * v + 1)
            vlast = ARRIVAL[-1]
            mul_piece(B - 1, xb, vlast, NSUB)
            mm(B - 1, 2 * vlast)
            drain_half(0)
            mm(B - 1, 2 * vlast + 1)
            drain_half(1)

    nc.finalize()  # runs Bacc.compile(): reg alloc + multi-wait lowering
    return nc


def _ones_blk():
    if "ones_blk" not in _cache:
        o = np.zeros((128, B, 32), dtype=np.float32)
        p = np.arange(128)
        for b in range(B):
            o[p, b, 2 * b + (p // 64)] = 1.0
        _cache["ones_blk"] = np.ascontiguousarray(o.reshape(128, B * 32).astype(np.float16))
    return _cache["ones_blk"]


def _ones_rep():
    if "ones_rep" not in _cache:
        o = np.zeros((128, 32), dtype=np.float32)
        p = np.arange(128)[:, None]
        m = np.arange(32)[None, :]
        o[(m % 2) == (p // 64)] = 1.0
        _cache["ones_rep"] = np.ascontiguousarray(o.astype(np.float16))
    return _cache["ones_rep"]


def _ones_bias():
    if "ones_bias" not in _cache:
        o = np.zeros((2, 32), dtype=np.float32)
        h = np.arange(2)[:, None]
        m = np.arange(32)[None, :]
        o[(m % 2) == h] = 1.0
        _cache["ones_bias"] = np.ascontiguousarray(o.astype(np.float16))
    return _cache["ones_bias"]


def _shard_inputs(x, weight, bias, with_bias):
    """Host-side (layout only) sharding/packing. Returns list of 8 input maps."""
    # fp16 staging (pure dtype cast, done once before the per-core loop
    # so the transposes below move half the bytes)
    x = np.asarray(x, dtype=np.float32).astype(np.float16)
    weight = np.asarray(weight, dtype=np.float32)
    bias = np.asarray(bias, dtype=np.float32)
    w_mat = weight.reshape(H * W, C).astype(np.float16)
    bias_flat = bias.reshape(H * W)

    in_maps = []
    for k in range(NCORES):
        r0 = k * ROWS
        xs = x[:, :, r0:r0 + ROWS, :].reshape(B, C, PIX)
        # [B, C, 2, HALF] -> [B, 2, C, HALF] -> [B, 128, HALF]
        x_core = np.ascontiguousarray(
            xs.reshape(B, C, 2, HALF).transpose(0, 2, 1, 3)
        ).reshape(B, 128, HALF)

        ws = w_mat[r0 * W:(r0 + ROWS) * W, :]          # [PIX, C]
        # -> [2, HALF, C] -> [2, C, HALF] -> [128, HALF]
        w_core = np.ascontiguousarray(
            ws.reshape(2, HALF, C).transpose(0, 2, 1)
        ).reshape(128, HALF)

        m = {
            "x": x_core,
            "w": w_core,
            "ones_blk": _ones_blk(),
            "ones_rep": _ones_rep(),
        }
        if with_bias:
            # [2, NSUB, SUB] -> replicate over b -> row 32s + 2b + h
            v = bias_flat[r0 * W:(r0 + ROWS) * W].reshape(2, NSUB, SUB)
            bl = np.broadcast_to(v[None], (B, 2, NSUB, SUB))
            m["bias"] = np.ascontiguousarray(
                bl.transpose(2, 0, 1, 3).reshape(128, SUB)).astype(np.float16)
        in_maps.append(m)
    return in_maps


def _unshard_output(results):
    out = np.zeros((B, 1, H, W), dtype=np.float32)
    for k in range(NCORES):
        # device layout: partition 32s + 2b + h holds columns s*SUB..(s+1)*SUB
        r = np.asarray(results[k]["out"], dtype=np.float32)   # [128, SUB]
        r = r.reshape(NSUB, B, 2, SUB).transpose(1, 2, 0, 3).reshape(B, PIX)
        out[:, 0, k * ROWS:(k + 1) * ROWS, :] = r.reshape(B, ROWS, W)
    return out


def _install_ntff_hook_shim():
    """This image lacks antenv.axon_hooks; bass_utils imports it whenever
    tracing is requested (including via a BASS_TRACE env var).  Recreate it
    with the ctypes-based hook from trn_boot so tracing degrades gracefully
    instead of crashing.  Idempotent and silent."""
    import sys
    try:
        import antenv.axon_hooks  # noqa: F401
        return
    except ImportError:
        pass
    try:
        import contextlib
        import ctypes
        import types

        mod = types.ModuleType("antenv.axon_hooks")
        state = {"hook": None}
        mod.set_axon_ntff_profile_hook = lambda h: state.__setitem__("hook", h)
        mod.get_axon_ntff_profile_hook = lambda: state["hook"]
        sys.modules["antenv.axon_hooks"] = mod

        so_path = "/opt/axon/libaxon_pjrt.so"
        lib = ctypes.CDLL(so_path)
        if not hasattr(lib, "axon_start_nrt_profile"):
            return
        lib.axon_start_nrt_profile.argtypes = [
            ctypes.POINTER(ctypes.c_int64), ctypes.c_size_t]
        lib.axon_start_nrt_profile.restype = ctypes.c_int64
        lib.axon_stop_nrt_profile.argtypes = [ctypes.c_char_p]
        lib.axon_stop_nrt_profile.restype = ctypes.c_int64

        @contextlib.contextmanager
        def _hook(output_dir, device_ids):
            import jax

            jax.devices()
            if device_ids:
                ids = (ctypes.c_int64 * len(device_ids))(*device_ids)
                rc = lib.axon_start_nrt_profile(ids, len(device_ids))
            else:
                rc = lib.axon_start_nrt_profile(None, 0)
            if rc != 0:
                raise RuntimeError(f"axon_start_nrt_profile rc={rc}")
            try:
                yield
            finally:
                lib.axon_stop_nrt_profile(str(output_dir).encode())

        mod.set_axon_ntff_profile_hook(_hook)
    except Exception:
        pass


def _run(inputs, trace=False, use_f32r=True):
    _ensure_jax_platform()
    _install_ntff_hook_shim()
    import concourse.bass_utils as _bu
    from concourse.bass_utils import run_bass_kernel_spmd

    # no cloud bucket in this container; keep trace artifacts local
    _bu.upload_artifacts = lambda tmpdir: tmpdir

    with_bias = bool(np.any(np.asarray(inputs["bias"])))
    key = ("nc", use_f32r, with_bias)
    if key not in _cache:
        _cache[key] = _build_nc(use_f32r=use_f32r, with_bias=with_bias)
    nc = _cache[key]

    in_maps = _shard_inputs(inputs["x"], inputs["weight"], inputs["bias"],
                            with_bias)
    res = run_bass_kernel_spmd(
        nc, in_maps, core_ids=list(range(NCORES)), trace=trace
    )
    return _unshard_output(res.results), res


def kernel(x, weight, bias):
    out, _ = _run({"x": x, "weight": weight, "bias": bias})
    return out

